# revision 12
# baseline (speedup 1.0000x reference)
"""Multi-head attention (B=4, S=2048, E=1024, 16 heads x 64) on 8 Trainium2 cores.

Sharding: core c = 2*b + half handles batch b and heads [8*half, 8*half+8)
(embed slice [512*half, 512*half+512)).  Each core computes its Q/K/V
projections, 8 heads of attention, and a row-parallel out-projection partial
(2048, 1024).  Host unshard: out[b] = partial[2b] + partial[2b+1] + bo.

Device kernel design (v2 — ACT-saturating pipeline):
  - The softmax exp on the Scalar engine is the hard floor (~33.5M elems
    at 1 elem/cycle/lane).  The whole kernel is one software pipeline that
    keeps ACT busy with [128, 2048] exp instructions (2 kt tiles per
    instruction to amortize the ~352-cycle ACT overhead).
  - PSUM: a 6-bank ring ([128, 3, 1024] fp32) shared by energy matmuls and
    (stolen slots) projection / out-projection accumulations, plus 2 banks
    for the two attn@V accumulators [65, 512].
  - Energies for the head pair (2m, 2m+1) are computed by two concurrent
    64-row-tile matmuls (tile_position (0,0) / (64,0)) — 2x PE throughput
    on the d=64 contraction.
  - exp pairs two ring slots with a strided [128, 2, 1024] AP (always
    ascending slot order; kt order per pair is tracked host-side).
  - V carries a prepended ones column (col 0) so attn@V yields the softmax
    denominator in PSUM row 0; normalization = reciprocal_approx_fast on
    row 0 (DVE), gpsimd partition_broadcast, multiply-on-evict.
  - Q/K/V projections and the out-projection are woven into the attention
    stream as "steal tasks" (1 ring slot each, ~8 matmuls), paced at most
    one per kt-pair so energies/exp never starve.
"""

import numpy as np
import ml_dtypes

import concourse.bass as bass
import concourse.mybir as mybir
import concourse.tile as tile
import concourse.bacc as bacc
from concourse.bass_utils import run_bass_kernel_spmd

BF16 = mybir.dt.bfloat16
F32 = mybir.dt.float32
NPBF = ml_dtypes.bfloat16

S = 2048          # sequence length
E = 1024          # embed dim
DLOC = 512        # per-core embed slice (8 heads x 64)
HD = 64           # head dim
NHL = 8           # heads per core
KT = E // 128     # 8 contraction tiles for projections
MT = DLOC // 128  # 4 m-tiles of d_local (= head pairs)
ST = S // 128     # 16 seq tiles (kt)
NCH = S // 512    # 4 seq chunks of 512
QC = S // 512     # 4 query chunks of 512
EXP = mybir.ActivationFunctionType.Exp
ADD = mybir.AluOpType.add
MULT = mybir.AluOpType.mult


def _build_bass(dump=False):
    nc = bacc.Bacc("TRN2", target_bir_lowering=False, debug=False)

    xqT = nc.dram_tensor("xqT", [E, S], BF16, kind="ExternalInput").ap()
    xkT = nc.dram_tensor("xkT", [E, S], BF16, kind="ExternalInput").ap()
    xvT = nc.dram_tensor("xvT", [E, S], BF16, kind="ExternalInput").ap()
    wq_d = nc.dram_tensor("wq", [E, DLOC], BF16, kind="ExternalInput").ap()
    wk_d = nc.dram_tensor("wk", [E, DLOC], BF16, kind="ExternalInput").ap()
    wv_d = nc.dram_tensor("wv", [E, DLOC], BF16, kind="ExternalInput").ap()
    wo_d = nc.dram_tensor("wo", [DLOC, E], BF16, kind="ExternalInput").ap()
    bq_d = nc.dram_tensor("bq", [128, MT], F32, kind="ExternalInput").ap()
    bk_d = nc.dram_tensor("bk", [128, MT], F32, kind="ExternalInput").ap()
    bv_d = nc.dram_tensor("bv", [1, DLOC], F32, kind="ExternalInput").ap()
    out_d = nc.dram_tensor("out", [S, E], F32, kind="ExternalOutput").ap()

    xq_r = xqT.rearrange("(kt p) s -> p kt s", p=128)
    xk_r = xkT.rearrange("(kt p) s -> p kt s", p=128)
    xv_r = xvT.rearrange("(kt p) s -> p kt s", p=128)

    with tile.TileContext(nc) as tc:
        _kernel_body(tc, nc, xq_r, xk_r, xv_r, wq_d, wk_d, wv_d, wo_d,
                     bq_d, bk_d, bv_d, out_d, dump=dump)
    nc.compile()
    return nc


def _kernel_body(tc, nc, xq_r, xk_r, xv_r, wq_d, wk_d, wv_d, wo_d,
                 bq_d, bk_d, bv_d, out_d, dump=False):
    from contextlib import ExitStack

    with ExitStack() as ctx:
        wpool = ctx.enter_context(tc.tile_pool(name="weights", bufs=1))
        xpool = ctx.enter_context(tc.tile_pool(name="xstream", bufs=3))
        qkv = ctx.enter_context(tc.tile_pool(name="qkv", bufs=1))
        atp = ctx.enter_context(tc.tile_pool(name="attnt", bufs=8))
        stp = ctx.enter_context(tc.tile_pool(name="stage", bufs=3))
        smp = ctx.enter_context(tc.tile_pool(name="small", bufs=2))
        outp = ctx.enter_context(tc.tile_pool(name="outstage", bufs=3))

        # ---- weights / biases to SBUF ----
        wq_sb = wpool.tile([128, KT, DLOC], BF16)
        wk_sb = wpool.tile([128, KT, DLOC], BF16)
        wv_sb = wpool.tile([128, KT, DLOC], BF16)
        wo_sb = wpool.tile([128, MT, E], BF16)
        bq_sb = wpool.tile([128, MT], F32)
        bk_sb = wpool.tile([128, MT], F32)
        bv_row = wpool.tile([1, DLOC], F32)
        bv_bc = wpool.tile([128, DLOC], F32)
        nc.sync.dma_start(wk_sb[:], wk_d.rearrange("(kt p) m -> p kt m", p=128))
        nc.sync.dma_start(wq_sb[:], wq_d.rearrange("(kt p) m -> p kt m", p=128))
        nc.sync.dma_start(wv_sb[:], wv_d.rearrange("(kt p) m -> p kt m", p=128))
        nc.sync.dma_start(wo_sb[:], wo_d.rearrange("(mt p) e -> p mt e", p=128))
        nc.sync.dma_start(bq_sb[:], bq_d)
        nc.sync.dma_start(bk_sb[:], bk_d)
        nc.sync.dma_start(bv_row[:], bv_d)
        nc.gpsimd.partition_broadcast(bv_bc[:], bv_row[:])

        # ---- persistent per-core tensors ----
        QT_sb = qkv.tile([128, MT, S], BF16)        # [d_loc, seq]
        KT_sb = qkv.tile([128, MT, S], BF16)
        V_sb = qkv.tile([128, ST, NHL, HD + 1], BF16)  # ones col at 64
        oT_sb = qkv.tile([128, MT, S], BF16)        # attn out^T (lhsT of outproj)

        nc.vector.memset(V_sb[:, :, :, HD:HD + 1], 1.0)

        # ---- PSUM: 6-bank ring + 2 attn@V accumulator banks ----
        pring = ctx.enter_context(tc.tile_pool(name="psum_ring", bufs=1, space="PSUM"))
        ppo = ctx.enter_context(tc.tile_pool(name="psum_po", bufs=1, space="PSUM"))
        ring = pring.tile([128, 3, 1024], F32)
        po0 = ppo.tile([HD + 1, 512], F32)
        po1 = ppo.tile([HD + 1, 512], F32)

        ring_ptr = [0]

        def take_slot():
            s = ring_ptr[0]
            ring_ptr[0] = (s + 1) % 3
            return s

        # ---------- steal tasks ----------
        # Each task: one ring slot, two [128,512] psum halves (A = kt 0-3,
        # B = kt 4-7 of the contraction), evicted with 2 DVE ops.
        def qk_task(t, m, nch):
            x_r = (xq_r, xk_r)[t]
            w_sb = (wq_sb, wk_sb)[t]
            b_sb = (bq_sb, bk_sb)[t]
            dst = (QT_sb, KT_sb)[t]
            x_t = xpool.tile([128, KT, 512], BF16, tag="xs", name="x_t")
            nc.sync.dma_start(x_t[:], x_r[:, :, bass.ts(nch, 512)])
            s = take_slot()
            psA = ring[:, s, 0:512]
            psB = ring[:, s, 512:1024]
            for kt in range(4):
                nc.tensor.matmul(psA, w_sb[:, kt, bass.ts(m, 128)],
                                 x_t[:, kt, :], start=(kt == 0), stop=(kt == 3))
            for kt in range(4, KT):
                nc.tensor.matmul(psB, w_sb[:, kt, bass.ts(m, 128)],
                                 x_t[:, kt, :], start=(kt == 4), stop=(kt == KT - 1))
            stage = stp.tile([128, 512], F32, tag="st")
            nc.vector.tensor_copy(stage[:], psA)
            nc.vector.scalar_tensor_tensor(
                dst[:, m, bass.ts(nch, 512)], psB, b_sb[:, m:m + 1], stage[:],
                ADD, ADD)

        def v_task(x_t, nch, stl):
            st = nch * 4 + stl
            s = take_slot()
            psA = ring[:, s, 0:512]
            psB = ring[:, s, 512:1024]
            for kt in range(4):
                nc.tensor.matmul(psA, x_t[:, kt, bass.ts(stl, 128)],
                                 wv_sb[:, kt, :], start=(kt == 0), stop=(kt == 3))
            for kt in range(4, KT):
                nc.tensor.matmul(psB, x_t[:, kt, bass.ts(stl, 128)],
                                 wv_sb[:, kt, :], start=(kt == 4), stop=(kt == KT - 1))
            stage = stp.tile([128, 512], F32, tag="st")
            nc.vector.tensor_tensor(stage[:], psA, bv_bc[:], ADD)
            nc.vector.tensor_tensor(
                V_sb[:, st, :, 0:HD],
                psB.rearrange("p (h d) -> p h d", d=HD),
                stage.rearrange("p (h d) -> p h d", d=HD), ADD)

        def out_task(qt):
            s = take_slot()
            for ec in range(2):
                ps = ring[:, s, bass.ts(ec, 512)]
                for m in range(MT):
                    nc.tensor.matmul(ps, oT_sb[:, m, bass.ts(qt, 128)],
                                     wo_sb[:, m, bass.ts(ec, 512)],
                                     start=(m == 0), stop=(m == MT - 1))
            ob = outp.tile([128, E], F32, tag="ob")
            nc.vector.tensor_copy(ob[:], ring[:, s, :])
            nc.sync.dma_start(out_d[bass.ts(qt, 128), :], ob[:])

        # V super-task bookkeeping: one x DMA per nch, shared by 4 stl tasks.
        v_x = {}
        # emission watermarks (build-time scheduling safety)
        qk_done = set()            # (t, m, nch) emitted
        v_done = set()             # st emitted

        def v_task_wrap(nch, stl):
            if stl == 0:
                x_t = xpool.tile([128, KT, 512], BF16, tag="xs", name="xv_t")
                nc.sync.dma_start(x_t[:], xv_r[:, :, bass.ts(nch, 512)])
                v_x[nch] = x_t
            v_task(v_x[nch], nch, stl)
            v_done.add(nch * 4 + stl)

        def qk_task_wrap(t, m, nch):
            qk_task(t, m, nch)
            qk_done.add((t, m, nch))

        # ---------- prologue ----------
        qk_task_wrap(1, 0, 0)      # K m0 chunk 0
        qk_task_wrap(0, 0, 0)      # Q m0 chunk 0

        # Weave queue (beyond prologue), in emission order.  All V tasks
        # come early: attn@V reads must be EMITTED after their V writers.
        weave = []
        weave += [("qk", 1, 0, n) for n in (1, 2, 3)]          # K m0 rest
        weave += [("v", 0, s2) for s2 in range(4)]
        weave += [("qk", 0, 0, 1)]                              # Q m0 n1
        weave += [("v", 1, s2) for s2 in range(4)]
        weave += [("v", 2, s2) for s2 in range(4)]
        weave += [("qk", 0, 0, 2)]                              # Q m0 n2
        weave += [("v", 3, s2) for s2 in range(4)]
        weave += [("qk", 0, 0, 3)]                              # Q m0 n3
        for m in (1, 2, 3):
            for n in range(NCH):
                weave += [("qk", 1, m, n), ("qk", 0, m, n)]
        # out-projection tasks are appended dynamically after (m3, qc) combos

        wq_i = [0]

        def pop_weave(k=1):
            for _ in range(k):
                if wq_i[0] >= len(weave):
                    return
                task = weave[wq_i[0]]
                wq_i[0] += 1
                if task[0] == "qk":
                    qk_task_wrap(*task[1:])
                else:
                    v_task_wrap(*task[1:])

        def v_ready(kmap):
            return all(kt in v_done for kt, _ in kmap)

        def flush_v(upto_st):
            while upto_st not in v_done and wq_i[0] < len(weave):
                pop_weave(1)

        # ---------- attention ----------
        # Pacing: per kt-pair emit [due attnV] [E,E] [exp] [steal(s)].
        # attnV for a pair is emitted once its V tiles have been emitted;
        # remaining V projections are force-flushed at combo end.
        first_combo = [True]
        for m in range(MT):
            h0, h1 = 2 * m, 2 * m + 1
            for qc in range(QC):
                q0 = qc * 512
                pending = []   # (at_tile, [(kt, col_base), ...])
                n_av = [0]     # attn@V emission counter (start/stop flags)

                def attn_v(pkmap, pat):
                    for kt, cb in pkmap:
                        assert kt in v_done
                        st_f, sp_f = n_av[0] == 0, n_av[0] == ST - 1
                        nc.tensor.matmul(
                            po0[:], V_sb[:, kt, h0, :], pat[:, cb:cb + 512],
                            start=st_f, stop=sp_f)
                        nc.tensor.matmul(
                            po1[:], V_sb[:, kt, h1, :],
                            pat[:, cb + 512:cb + 1024],
                            start=st_f, stop=sp_f)
                        n_av[0] += 1

                for j in range(8):
                    kta, ktb = 2 * j, 2 * j + 1
                    assert (1, m, kta // 4) in qk_done, (m, qc, kta)
                    assert (1, m, ktb // 4) in qk_done, (m, qc, ktb)
                    assert (0, m, qc) in qk_done, (m, qc)
                    # energies: head pair concurrent on row tiles (0,0)/(64,0)
                    sa = take_slot()
                    for (psl, tp, c0) in (((0, 64), (0, 0), 0),
                                          ((64, 128), (64, 0), 512)):
                        nc.tensor.matmul(
                            ring[:, sa, c0:c0 + 512],
                            KT_sb[psl[0]:psl[1], m, bass.ts(kta, 128)],
                            QT_sb[psl[0]:psl[1], m, bass.ds(q0, 512)],
                            start=True, stop=True, tile_position=tp)
                    sb = take_slot()
                    for (psl, tp, c0) in (((0, 64), (0, 0), 0),
                                          ((64, 128), (64, 0), 512)):
                        nc.tensor.matmul(
                            ring[:, sb, c0:c0 + 512],
                            KT_sb[psl[0]:psl[1], m, bass.ts(ktb, 128)],
                            QT_sb[psl[0]:psl[1], m, bass.ds(q0, 512)],
                            start=True, stop=True, tile_position=tp)
                    # paired exp over both slots (ascending slot order)
                    lo, hi = (sa, sb) if sa < sb else (sb, sa)
                    at = atp.tile([128, 2048], BF16, tag="at")
                    nc.scalar.activation(
                        at.rearrange("p (t n) -> p t n", n=1024),
                        ring[:, lo:hi + 1:(hi - lo), :], EXP)
                    if sa < sb:
                        kmap = [(kta, 0), (ktb, 1024)]
                    else:
                        kmap = [(ktb, 0), (kta, 1024)]
                    pending.append((at, kmap))
                    pop_weave(2 if first_combo[0] else 1)
                    # attnV for pairs whose V projections are emitted
                    while pending and v_ready(pending[0][1]):
                        pat, pkmap = pending.pop(0)
                        attn_v(pkmap, pat)
                # force any remaining V projections, then drain attnV
                while pending:
                    flush_v(max(kt for kt, _ in pending[0][1]))
                    pat, pkmap = pending.pop(0)
                    attn_v(pkmap, pat)
                first_combo[0] = False

                # ---- normalize + evict (sums in PSUM row 0) ----
                for hs, po in ((0, po0), (1, po1)):
                    ssum = smp.tile([1, 512], F32, tag="ss")
                    nc.vector.tensor_copy(ssum[0:1, :], po[HD:HD + 1, :])
                    rs = smp.tile([1, 512], F32, tag="rs")
                    nc.vector.reciprocal_approx_fast(rs[0:1, :], ssum[0:1, :])
                    bc = smp.tile([HD, 512], F32, tag="bc")
                    nc.gpsimd.partition_broadcast(bc[:], rs[0:1, :])
                    nc.vector.tensor_tensor(
                        oT_sb[64 * hs:64 * hs + HD, m, bass.ds(q0, 512)],
                        po[0:HD, :], bc[:], MULT)

                # out-projection for completed q chunks (needs all m)
                if m == MT - 1 and qc > 0:
                    for qt in range(4 * (qc - 1), 4 * qc):
                        out_task(qt)

        # ---------- tail: remaining weave + last q-chunk out-projection ----
        pop_weave(len(weave))
        for qt in range(4 * (QC - 1), 4 * QC):
            out_task(qt)

        if dump:
            d_qt = nc.dram_tensor("d_qt", [128, MT, S], BF16, kind="ExternalOutput").ap()
            d_kt = nc.dram_tensor("d_kt", [128, MT, S], BF16, kind="ExternalOutput").ap()
            d_v = nc.dram_tensor("d_v", [128, ST, NHL, HD + 1], BF16, kind="ExternalOutput").ap()
            d_ot = nc.dram_tensor("d_ot", [128, MT, S], BF16, kind="ExternalOutput").ap()
            nc.sync.dma_start(d_qt, QT_sb[:])
            nc.sync.dma_start(d_kt, KT_sb[:])
            nc.sync.dma_start(d_v, V_sb[:])
            nc.sync.dma_start(d_ot, oT_sb[:])


_CACHED = {}


def _get_bass():
    if "nc" not in _CACHED:
        _CACHED["nc"] = _build_bass()
    return _CACHED["nc"]


def _prep_core_inputs(c, query, key, value, Wq, bq, Wk, bk, Wv, bv, Wo):
    b, half = c // 2, c % 2
    sl = slice(DLOC * half, DLOC * half + DLOC)
    bq_sl = (bq[sl] * 0.125).astype(np.float32).reshape(MT, 128).T.copy()
    bk_sl = bk[sl].astype(np.float32).reshape(MT, 128).T.copy()
    return {
        "xqT": np.ascontiguousarray(query[b].T).astype(NPBF),
        "xkT": np.ascontiguousarray(key[b].T).astype(NPBF),
        "xvT": np.ascontiguousarray(value[b].T).astype(NPBF),
        "wq": np.ascontiguousarray(Wq[sl, :].T * 0.125).astype(NPBF),
        "wk": np.ascontiguousarray(Wk[sl, :].T).astype(NPBF),
        "wv": np.ascontiguousarray(Wv[sl, :].T).astype(NPBF),
        "wo": np.ascontiguousarray(Wo[:, sl].T).astype(NPBF),
        "bq": np.ascontiguousarray(bq_sl),
        "bk": np.ascontiguousarray(bk_sl),
        "bv": bv[sl].astype(np.float32).reshape(1, DLOC).copy(),
    }


def kernel(query, key, value, Wq, bq, Wk, bk, Wv, bv, Wo, bo,
           trace=False, **run_kwargs):
    query = np.asarray(query, np.float32)
    key = np.asarray(key, np.float32)
    value = np.asarray(value, np.float32)
    Wq, Wk, Wv, Wo = (np.asarray(w, np.float32) for w in (Wq, Wk, Wv, Wo))
    bq, bk, bv, bo = (np.asarray(x, np.float32) for x in (bq, bk, bv, bo))

    nc = _get_bass()
    in_maps = [_prep_core_inputs(c, query, key, value, Wq, bq, Wk, bk, Wv, bv, Wo)
               for c in range(8)]
    res = run_bass_kernel_spmd(nc, in_maps, core_ids=list(range(8)),
                               trace=trace, **run_kwargs)
    _CACHED["last_result"] = res

    B = query.shape[0]
    out = np.empty((B, S, E), np.float32)
    for b in range(B):
        out[b] = res.results[2 * b]["out"] + res.results[2 * b + 1]["out"] + bo
    return out


# revision 15
# speedup vs baseline: 1.7920x; 1.7920x over previous
"""Multi-head attention (B=4, S=2048, E=1024, 16 heads x 64) on 8 Trainium2 cores.

Sharding: core c = 2*b + half handles batch b and heads [8*half, 8*half+8)
(embed slice [512*half, 512*half+512)).  Each core computes its Q/K/V
projections, 8 heads of attention, and a row-parallel out-projection partial
(2048, 1024).  Host unshard: out[b] = partial[2b] + partial[2b+1] + bo.

Device kernel design (v3 — ACT-saturating pipeline):
  - The softmax exp on the Scalar engine is the hard floor (~33.5M elems
    at 1 elem/cycle/lane, ~1147ns per [128,1024] tile).  The kernel is one
    software pipeline that keeps ACT running back-to-back exps.
  - PSUM (8 banks): pe0/pe1 [128,1024] energy double-buffer, a DEDICATED
    steal slot [128,1024] for projection/out-projection accumulations
    (so their matmuls + DVE evictions never sit on the exp critical path),
    and two attn@V accumulators [65,512].
  - Energies for the head pair (2m, 2m+1) are computed by two concurrent
    64-row-tile matmuls (tile_position (0,0)/(64,0), h0 -> cols 0:512,
    h1 -> cols 512:1024) — 2x PE throughput on the d=64 contraction.
  - V carries an appended ones column (col 64) so attn@V yields the softmax
    denominator in PSUM row 64; normalization = copy to SBUF (the custom-DVE
    reciprocal misreads PSUM on HW), reciprocal_approx_fast, gpsimd
    partition_broadcast, multiply-on-evict.
  - Q/K/V projections and the out-projection are woven between attention
    kts as [128,512] "steal" groups (8 matmuls + 1 DVE eviction each),
    paced so energies/exp never starve.
"""

import numpy as np
import ml_dtypes

import concourse.bass as bass
import concourse.mybir as mybir
import concourse.tile as tile
import concourse.bacc as bacc
from concourse.bass_utils import run_bass_kernel_spmd

BF16 = mybir.dt.bfloat16
F32 = mybir.dt.float32
NPBF = ml_dtypes.bfloat16

S = 2048          # sequence length
E = 1024          # embed dim
DLOC = 512        # per-core embed slice (8 heads x 64)
HD = 64           # head dim
NHL = 8           # heads per core
KT = E // 128     # 8 contraction tiles for projections
MT = DLOC // 128  # 4 m-tiles of d_local (= head pairs)
ST = S // 128     # 16 seq tiles (kt)
NCH = S // 512    # 4 seq chunks of 512
QC = S // 512     # 4 query chunks of 512
EXP = mybir.ActivationFunctionType.Exp
ADD = mybir.AluOpType.add
MULT = mybir.AluOpType.mult


def _build_bass(dump=False):
    nc = bacc.Bacc("TRN2", target_bir_lowering=False, debug=False)

    xqT = nc.dram_tensor("xqT", [E, S], BF16, kind="ExternalInput").ap()
    xkT = nc.dram_tensor("xkT", [E, S], BF16, kind="ExternalInput").ap()
    xvT = nc.dram_tensor("xvT", [E, S], BF16, kind="ExternalInput").ap()
    wq_d = nc.dram_tensor("wq", [E, DLOC], BF16, kind="ExternalInput").ap()
    wk_d = nc.dram_tensor("wk", [E, DLOC], BF16, kind="ExternalInput").ap()
    wv_d = nc.dram_tensor("wv", [E, DLOC], BF16, kind="ExternalInput").ap()
    wo_d = nc.dram_tensor("wo", [DLOC, E], BF16, kind="ExternalInput").ap()
    bq_d = nc.dram_tensor("bq", [128, MT], F32, kind="ExternalInput").ap()
    bk_d = nc.dram_tensor("bk", [128, MT], F32, kind="ExternalInput").ap()
    bv_d = nc.dram_tensor("bv", [1, DLOC], F32, kind="ExternalInput").ap()
    out_d = nc.dram_tensor("out", [S, E], F32, kind="ExternalOutput").ap()

    xq_r = xqT.rearrange("(kt p) s -> p kt s", p=128)
    xk_r = xkT.rearrange("(kt p) s -> p kt s", p=128)
    xv_r = xvT.rearrange("(kt p) s -> p kt s", p=128)

    with tile.TileContext(nc) as tc:
        _kernel_body(tc, nc, xq_r, xk_r, xv_r, wq_d, wk_d, wv_d, wo_d,
                     bq_d, bk_d, bv_d, out_d, dump=dump)
    nc.compile()
    return nc


def _kernel_body(tc, nc, xq_r, xk_r, xv_r, wq_d, wk_d, wv_d, wo_d,
                 bq_d, bk_d, bv_d, out_d, dump=False):
    from contextlib import ExitStack

    with ExitStack() as ctx:
        wpool = ctx.enter_context(tc.tile_pool(name="weights", bufs=1))
        xpool = ctx.enter_context(tc.tile_pool(name="xstream", bufs=3))
        qkv = ctx.enter_context(tc.tile_pool(name="qkv", bufs=1))
        atp = ctx.enter_context(tc.tile_pool(name="attnt", bufs=12))
        smp = ctx.enter_context(tc.tile_pool(name="small", bufs=2))
        outp = ctx.enter_context(tc.tile_pool(name="outstage", bufs=3))

        # ---- weights / biases to SBUF ----
        wq_sb = wpool.tile([128, KT, DLOC], BF16)
        wk_sb = wpool.tile([128, KT, DLOC], BF16)
        wv_sb = wpool.tile([128, KT, DLOC], BF16)
        wo_sb = wpool.tile([128, MT, E], BF16)
        bq_sb = wpool.tile([128, MT], F32)
        bk_sb = wpool.tile([128, MT], F32)
        bv_row = wpool.tile([1, DLOC], F32)
        bv_bc = wpool.tile([128, DLOC], F32)
        nc.sync.dma_start(wk_sb[:], wk_d.rearrange("(kt p) m -> p kt m", p=128))
        nc.sync.dma_start(wq_sb[:], wq_d.rearrange("(kt p) m -> p kt m", p=128))
        nc.sync.dma_start(wv_sb[:], wv_d.rearrange("(kt p) m -> p kt m", p=128))
        nc.sync.dma_start(wo_sb[:], wo_d.rearrange("(mt p) e -> p mt e", p=128))
        nc.sync.dma_start(bq_sb[:], bq_d)
        nc.sync.dma_start(bk_sb[:], bk_d)
        nc.sync.dma_start(bv_row[:], bv_d)
        nc.gpsimd.partition_broadcast(bv_bc[:], bv_row[:])

        # ---- persistent per-core tensors ----
        QT_sb = qkv.tile([128, MT, S], BF16)        # [d_loc, seq]
        KT_sb = qkv.tile([128, MT, S], BF16)
        V_sb = qkv.tile([128, ST, NHL, HD + 1], BF16)  # ones col at 64
        oT_sb = qkv.tile([128, MT, S], BF16)        # attn out^T (lhsT of outproj)

        nc.vector.memset(V_sb[:, :, :, HD:HD + 1], 1.0)

        # ---- PSUM: 2 energy slots + dedicated steal slot + attn@V accs ----
        pring = ctx.enter_context(tc.tile_pool(name="psum_ring", bufs=1, space="PSUM"))
        ppo = ctx.enter_context(tc.tile_pool(name="psum_po", bufs=1, space="PSUM"))
        pe = [pring.tile([128, 1024], F32, name="pe0"),
              pring.tile([128, 1024], F32, name="pe1")]
        psteal = pring.tile([128, 1024], F32, name="psteal")
        po0 = ppo.tile([HD + 1, 512], F32)
        po1 = ppo.tile([HD + 1, 512], F32)

        steal_half = [0]

        def take_half():
            h = steal_half[0]
            steal_half[0] ^= 1
            return psteal[:, h * 512:h * 512 + 512]

        # ---------- steal tasks ----------
        # Each task: one [128,512] half of the steal slot, 8 K-accumulating
        # matmuls, one DVE eviction.
        def qk_task(t, m, nch):
            x_r = (xq_r, xk_r)[t]
            w_sb = (wq_sb, wk_sb)[t]
            b_sb = (bq_sb, bk_sb)[t]
            dst = (QT_sb, KT_sb)[t]
            x_t = xpool.tile([128, KT, 512], BF16, tag="xs", name="x_t")
            nc.sync.dma_start(x_t[:], x_r[:, :, bass.ts(nch, 512)])
            ps = take_half()
            for kt in range(KT):
                nc.tensor.matmul(ps, w_sb[:, kt, bass.ts(m, 128)],
                                 x_t[:, kt, :], start=(kt == 0), stop=(kt == KT - 1))
            nc.vector.tensor_scalar_add(
                dst[:, m, bass.ts(nch, 512)], ps, b_sb[:, m:m + 1])

        def v_task(x_t, nch, stl):
            st = nch * 4 + stl
            ps = take_half()
            for kt in range(KT):
                nc.tensor.matmul(ps, x_t[:, kt, bass.ts(stl, 128)],
                                 wv_sb[:, kt, :], start=(kt == 0), stop=(kt == KT - 1))
            nc.vector.tensor_tensor(
                V_sb[:, st, :, 0:HD],
                ps.rearrange("p (h d) -> p h d", d=HD),
                bv_bc.rearrange("p (h d) -> p h d", d=HD), ADD)

        def out_task(qt):
            for ec in range(2):
                ps = take_half()
                for m in range(MT):
                    nc.tensor.matmul(ps, oT_sb[:, m, bass.ts(qt, 128)],
                                     wo_sb[:, m, bass.ts(ec, 512)],
                                     start=(m == 0), stop=(m == MT - 1))
                ob = outp.tile([128, 512], F32, tag="ob")
                nc.vector.tensor_copy(ob[:], ps)
                nc.sync.dma_start(
                    out_d[bass.ts(qt, 128), bass.ts(ec, 512)], ob[:])

        # V super-task bookkeeping: one x DMA per nch, shared by 4 stl tasks.
        v_x = {}
        # emission watermarks (build-time scheduling safety)
        qk_done = set()            # (t, m, nch) emitted
        v_done = set()             # st emitted

        def v_task_wrap(nch, stl):
            if stl == 0:
                x_t = xpool.tile([128, KT, 512], BF16, tag="xs", name="xv_t")
                nc.sync.dma_start(x_t[:], xv_r[:, :, bass.ts(nch, 512)])
                v_x[nch] = x_t
            v_task(v_x[nch], nch, stl)
            v_done.add(nch * 4 + stl)

        def qk_task_wrap(t, m, nch):
            qk_task(t, m, nch)
            qk_done.add((t, m, nch))

        # ---------- prologue ----------
        qk_task_wrap(1, 0, 0)      # K m0 chunk 0
        qk_task_wrap(0, 0, 0)      # Q m0 chunk 0

        # Weave queue (beyond prologue), in emission order.  All V tasks
        # come early: attn@V reads must be EMITTED after their V writers.
        weave = []
        weave += [("qk", 1, 0, n) for n in (1, 2, 3)]          # K m0 rest
        weave += [("v", 0, s2) for s2 in range(4)]
        weave += [("qk", 0, 0, 1)]                              # Q m0 n1
        weave += [("v", 1, s2) for s2 in range(4)]
        weave += [("v", 2, s2) for s2 in range(4)]
        weave += [("qk", 0, 0, 2)]                              # Q m0 n2
        weave += [("v", 3, s2) for s2 in range(4)]
        weave += [("qk", 0, 0, 3)]                              # Q m0 n3
        for m in (1, 2, 3):
            for n in range(NCH):
                weave += [("qk", 1, m, n), ("qk", 0, m, n)]
        # out-projection tasks are appended dynamically after (m3, qc) combos

        wq_i = [0]

        def pop_weave(k=1):
            for _ in range(k):
                if wq_i[0] >= len(weave):
                    return
                task = weave[wq_i[0]]
                wq_i[0] += 1
                if task[0] == "qk":
                    qk_task_wrap(*task[1:])
                else:
                    v_task_wrap(*task[1:])

        def flush_v(upto_st):
            while upto_st not in v_done and wq_i[0] < len(weave):
                pop_weave(1)

        # ---------- attention ----------
        # Per kt: [E -> pe[kt%2]] [exp] [due attnV] [steal every 2nd kt].
        # attnV for a kt is emitted once its V tile has been emitted;
        # remaining V projections are force-flushed at combo end.
        first_combo = [True]
        for m in range(MT):
            h0, h1 = 2 * m, 2 * m + 1
            for qc in range(QC):
                q0 = qc * 512
                pending = []   # (at_tile, kt)
                n_av = [0]     # attn@V emission counter (start/stop flags)

                def attn_v(pat, kt):
                    assert kt in v_done
                    st_f, sp_f = n_av[0] == 0, n_av[0] == ST - 1
                    nc.tensor.matmul(
                        po0[:], V_sb[:, kt, h0, :], pat[:, 0:512],
                        start=st_f, stop=sp_f)
                    nc.tensor.matmul(
                        po1[:], V_sb[:, kt, h1, :], pat[:, 512:1024],
                        start=st_f, stop=sp_f)
                    n_av[0] += 1

                for kt in range(ST):
                    assert (1, m, kt // 4) in qk_done, (m, qc, kt)
                    assert (0, m, qc) in qk_done, (m, qc)
                    # energies: head pair concurrent on row tiles (0,0)/(64,0)
                    ps = pe[kt % 2]
                    for (psl, tp, c0) in (((0, 64), (0, 0), 0),
                                          ((64, 128), (64, 0), 512)):
                        nc.tensor.matmul(
                            ps[:, c0:c0 + 512],
                            KT_sb[psl[0]:psl[1], m, bass.ts(kt, 128)],
                            QT_sb[psl[0]:psl[1], m, bass.ds(q0, 512)],
                            start=True, stop=True, tile_position=tp)
                    at = atp.tile([128, 1024], BF16, tag="at")
                    nc.scalar.activation(at[:], ps[:], EXP)
                    pending.append((at, kt))
                    # attnV for kts whose V projections are emitted
                    while pending and pending[0][1] in v_done:
                        pat, pkt = pending.pop(0)
                        attn_v(pat, pkt)
                    if kt % 2 == 1 or first_combo[0]:
                        pop_weave(1)
                # force any remaining V projections, then drain attnV
                while pending:
                    flush_v(pending[0][1])
                    pat, pkt = pending.pop(0)
                    attn_v(pat, pkt)
                first_combo[0] = False

                # ---- normalize + evict (sums in PSUM row 0) ----
                for hs, po in ((0, po0), (1, po1)):
                    ssum = smp.tile([1, 512], F32, tag="ss")
                    nc.vector.tensor_copy(ssum[0:1, :], po[HD:HD + 1, :])
                    rs = smp.tile([1, 512], F32, tag="rs")
                    nc.vector.reciprocal_approx_fast(rs[0:1, :], ssum[0:1, :])
                    bc = smp.tile([HD, 512], F32, tag="bc")
                    nc.gpsimd.partition_broadcast(bc[:], rs[0:1, :])
                    nc.vector.tensor_tensor(
                        oT_sb[64 * hs:64 * hs + HD, m, bass.ds(q0, 512)],
                        po[0:HD, :], bc[:], MULT)

                # out-projection for completed q chunks (needs all m)
                if m == MT - 1 and qc > 0:
                    for qt in range(4 * (qc - 1), 4 * qc):
                        out_task(qt)

        # ---------- tail: remaining weave + last q-chunk out-projection ----
        pop_weave(len(weave))
        for qt in range(4 * (QC - 1), 4 * QC):
            out_task(qt)

        if dump:
            d_qt = nc.dram_tensor("d_qt", [128, MT, S], BF16, kind="ExternalOutput").ap()
            d_kt = nc.dram_tensor("d_kt", [128, MT, S], BF16, kind="ExternalOutput").ap()
            d_v = nc.dram_tensor("d_v", [128, ST, NHL, HD + 1], BF16, kind="ExternalOutput").ap()
            d_ot = nc.dram_tensor("d_ot", [128, MT, S], BF16, kind="ExternalOutput").ap()
            nc.sync.dma_start(d_qt, QT_sb[:])
            nc.sync.dma_start(d_kt, KT_sb[:])
            nc.sync.dma_start(d_v, V_sb[:])
            nc.sync.dma_start(d_ot, oT_sb[:])


_CACHED = {}


def _get_bass():
    if "nc" not in _CACHED:
        _CACHED["nc"] = _build_bass()
    return _CACHED["nc"]


def _prep_core_inputs(c, query, key, value, Wq, bq, Wk, bk, Wv, bv, Wo):
    b, half = c // 2, c % 2
    sl = slice(DLOC * half, DLOC * half + DLOC)
    bq_sl = (bq[sl] * 0.125).astype(np.float32).reshape(MT, 128).T.copy()
    bk_sl = bk[sl].astype(np.float32).reshape(MT, 128).T.copy()
    return {
        "xqT": np.ascontiguousarray(query[b].T).astype(NPBF),
        "xkT": np.ascontiguousarray(key[b].T).astype(NPBF),
        "xvT": np.ascontiguousarray(value[b].T).astype(NPBF),
        "wq": np.ascontiguousarray(Wq[sl, :].T * 0.125).astype(NPBF),
        "wk": np.ascontiguousarray(Wk[sl, :].T).astype(NPBF),
        "wv": np.ascontiguousarray(Wv[sl, :].T).astype(NPBF),
        "wo": np.ascontiguousarray(Wo[:, sl].T).astype(NPBF),
        "bq": np.ascontiguousarray(bq_sl),
        "bk": np.ascontiguousarray(bk_sl),
        "bv": bv[sl].astype(np.float32).reshape(1, DLOC).copy(),
    }


def kernel(query, key, value, Wq, bq, Wk, bk, Wv, bv, Wo, bo,
           trace=False, **run_kwargs):
    query = np.asarray(query, np.float32)
    key = np.asarray(key, np.float32)
    value = np.asarray(value, np.float32)
    Wq, Wk, Wv, Wo = (np.asarray(w, np.float32) for w in (Wq, Wk, Wv, Wo))
    bq, bk, bv, bo = (np.asarray(x, np.float32) for x in (bq, bk, bv, bo))

    nc = _get_bass()
    in_maps = [_prep_core_inputs(c, query, key, value, Wq, bq, Wk, bk, Wv, bv, Wo)
               for c in range(8)]
    res = run_bass_kernel_spmd(nc, in_maps, core_ids=list(range(8)),
                               trace=trace, **run_kwargs)
    _CACHED["last_result"] = res

    B = query.shape[0]
    out = np.empty((B, S, E), np.float32)
    for b in range(B):
        out[b] = res.results[2 * b]["out"] + res.results[2 * b + 1]["out"] + bo
    return out
